# revision 38
# baseline (speedup 1.0000x reference)
"""CAML attention kernel for Trainium2 (8 NeuronCores, SPMD over classes).

Reference computation:
    xt      = tanh(x)                      # [B, D, L]
    scores  = einsum('cd,bdl->bcl', W1, xt)
    weights = softmax(scores, axis=l)
    weighted= einsum('bcl,bdl->bcd', weights, xt)
    out     = einsum('cd,bcd->bc', W2, weighted) + b2

Key identity: the final contraction commutes with the softmax weighted sum,
so with s2 = einsum('cd,bdl->bcl', W2, xt):
    out[b,c] = (sum_l exp(s1)*s2) / (sum_l exp(s1)) + b2
(|s1| <= 512*max|W1| ~ 13, so exp without max-subtraction is safe in fp32.)

v2: both matmuls run in fp8-e4m3 DoubleRow (2x PE throughput vs fp16 ->
~300us PE floor at 157 TF/s). Accuracy is recovered with a host-side
mean-correction folded into the bias:
    out ~= sum_l p_l s2q[c,l] + (W2 @ xbar_b - W2q @ xbar8_b)[c] + b2[c]
where xbar_b = mean_l tanh(x), xbar8_b = mean_l fp8(tanh(x)), both computed
on host (tiny C*D*B GEMM). The quantization error of W2 and of xt enters
out mostly through the (near-uniform-softmax) MEAN over L=2500 positions;
replacing that mean term with its exact value cuts rel err from ~1.5e-2 to
~6e-3 (sim), robust to device/host tanh table mismatch (which averages out
over L). The W1-side mean error cancels exactly via softmax shift
invariance, so no correction is needed there.

Other changes vs the fp16 baseline (638us -> 339us measured):
  - tanh (ACT) writes fp8 directly into the DoubleRow rhs layout
    [P, kch, lch, 512] (no DVE copy pass); x is shipped as fp8 (the extra
    tanh-input quantization costs ~1.5e-3 rel err, covered by the mean
    correction's margin, and shortens the cold-start x DMAs).
  - per-batch x loads + tanhs are spread across the previous batch's class
    loop (j in {1,3,5,7}), so ACT never bunches 9us of tanh at a batch
    boundary -- that bunching delayed exp, whose completion releases the
    s1 PSUM banks the PE needs next (measured 0.58us PE stalls + p-state
    recovery per occurrence before this change).
  - exp runs as 2 strided-AP instructions per (b,j): s1 PSUM tiles are
    (3,2) l-chunk groups (3+2 banks, single-buffered per tag -- the j->j+1
    serialization on exp has ~1.7us of slack), amortizing ACT PSUM-access
    latency and the 181ns accumulator-read of the fused denominator.
  - s2 stays at per-l 1-bank PSUM tiles (bufs=3) so the DVE product drains
    PSUM quickly: 5+3 = 8 banks exactly.
  - the softmax epilogue is batched per-b over all 9 class tiles ([P,9]
    ops) instead of per-(b,j) [P,1] ops.
  - one contiguous output DMA ([P, jch, b] layout) instead of 9 per-j
    DMAs, which serialized ~600ns of HWDGE setup each in the kernel tail.

Engine busy (pftrace, 339us total): PE 314us (93%, 1440 DoubleRow matmuls
in a zero-gap stream, LDWEIGHTS hidden by the PE reorder window), ACT
279us, DVE 240us. The PE matmul stream is the roofline; fp8-DR measures
2x bf16 (not 4x) on trn2. Remaining overhead is the ~7.7us runtime
preamble, the 9.1us serial batch-0 tanh chain on ACT (cold, fed at
~10.8us by the first fp8 x chunk - batch-0 x DMAs lead the queue, weights
ride under the tanh chain), and ~12us of tail/teardown; starting the PE
earlier via split/dual-queue cold loads was measured a wash (stalls just
move into the stream).

Sharding: C padded 8930 -> 9216 = 8 cores * 1152; weights row-sharded per
core, x replicated. Zero-padded weight rows give out=0 there, discarded on
the host after gathering.
"""

import numpy as np
import ml_dtypes

import concourse.bacc as bacc
import concourse.tile as tile
from concourse import mybir
from concourse.bass import ts
from concourse.bass_utils import run_bass_kernel_spmd

B, D, L, C = 8, 512, 2500, 8930
N_CORES = 8
P = 128

C_PAD = 9216                 # next multiple of 8*128 above C
C_SH = C_PAD // N_CORES      # 1152 classes per core
KCH = D // P                 # 4 contraction chunks (pairs for DoubleRow)
JCH = C_SH // P              # 9 class chunks per core
LCH = 5                      # l chunks
LT = L // LCH                # 500 columns per matmul (fits one PSUM bank)
LT8 = 512                    # l-chunk stride: bank-sized, 16B-aligned for fp8 rhs
LGROUPS = [(0, 1, 2), (3, 4)]  # l-chunks fused per s1-PSUM tile / exp instr

F32 = mybir.dt.float32
F16 = mybir.dt.float16
FP8 = mybir.dt.float8e4
FP8_NP = mybir.dt.np(mybir.dt.float8e4)  # ml_dtypes.float8_e4m3

# fp8 weights are scaled into e4m3's normal range (min normal 2^-6 vs
# |W| <= 0.0252); exp's input scale and the product's scalar compensate.
W1_SCALE = 16.0
W2_SCALE = 32.0

FP8_S1 = True  # legacy knob read by test.py; both paths are fp8 here


def build_nc(b=B, kch=KCH, jch=JCH):
    """Emit the per-core program. All cores run the same NEFF (SPMD)."""
    nc = bacc.Bacc("TRN2", target_bir_lowering=False, debug=False)

    x8 = nc.dram_tensor("x8", [b, kch, P, L], FP8, kind="ExternalInput")
    w1t = nc.dram_tensor("w1t", [kch, P, jch * P], FP8, kind="ExternalInput")
    w2t = nc.dram_tensor("w2t", [kch, P, jch * P], FP8, kind="ExternalInput")
    b2d = nc.dram_tensor("b2d", [P, jch, b], F32, kind="ExternalInput")
    out = nc.dram_tensor("out", [P, jch, b], F32, kind="ExternalOutput")

    Exp = mybir.ActivationFunctionType.Exp
    Tanh = mybir.ActivationFunctionType.Tanh
    mult = mybir.AluOpType.mult
    add = mybir.AluOpType.add
    DR = mybir.MatmulPerfMode.DoubleRow

    with tile.TileContext(nc) as tc:
        with (
            tc.tile_pool(name="wts", bufs=1) as wpool,
            tc.tile_pool(name="xraw", bufs=8) as xpool,
            tc.tile_pool(name="xt8", bufs=2) as xtpool,
            tc.tile_pool(name="ps1", bufs=1, space="PSUM") as ppool1,
            tc.tile_pool(name="ps2", bufs=3, space="PSUM") as ppool2,
            tc.tile_pool(name="etile", bufs=2) as epool,
            tc.tile_pool(name="scratch", bufs=3) as spool,
            tc.tile_pool(name="cols", bufs=2) as cpool,
            tc.tile_pool(name="epi", bufs=2) as eppool,
            tc.tile_pool(name="outp", bufs=1) as opool,
        ):
            # one fast HWDGE queue, ordered by first consumption
            w1sb = wpool.tile([P, kch, jch * P], FP8)
            w2sb = wpool.tile([P, kch, jch * P], FP8)
            b2sb = wpool.tile([P, jch, b], F32)
            out_all = opool.tile([P, jch, b], F32)

            # load fp8 x, tanh straight to fp8 in the DoubleRow rhs
            # layout [P, kch, LCH, LT8] (l-chunk stride 512 cols)
            xtbs = {}

            def emit_load(bload, k):
                xraw = xpool.tile([P, LCH, LT], FP8, tag="xraw")
                nc.sync.dma_start(out=xraw, in_=x8[bload, k])
                nc.scalar.activation(
                    out=xtbs[bload][:, k, :, 0:LT], in_=xraw, func=Tanh
                )

            # cold start: batch 0 up front; later batches prefetch spread
            # across the previous batch's class loop so ACT never bunches
            # tanhs at a batch boundary (which stalls the PE on PSUM reuse)
            xtbs[0] = xtpool.tile(
                [P, kch, LCH, LT8], FP8, tag="xt8", name="xtb"
            )
            # (starting the PE earlier via split/dual-queue cold loads was
            # measured a wash: the tanh-wait stalls just move into the
            # matmul stream -- the dependency-driven schedule below is
            # already cold-optimal given the 9.1us serial tanh chain)
            for k in range(kch):
                emit_load(0, k)
            for k in range(kch):
                nc.sync.dma_start(out=w1sb[:, k], in_=w1t[k])
            for k in range(kch):
                nc.sync.dma_start(out=w2sb[:, k], in_=w2t[k])
            nc.sync.dma_start(out=b2sb, in_=b2d[:])

            for bi in range(b):
                xtb = xtbs[bi]
                xraws = {}
                dcols = cpool.tile([P, 2 * jch], F32, tag="dcols")
                ncols = cpool.tile([P, LCH * jch], F32, tag="ncols")
                for j in range(jch):
                    # s1 tiles are single-buffered per group tag (3+2 banks);
                    # the j->j+1 matmul reuse serializes on exp(j), which
                    # finishes ~1.7us before the PE needs the banks back
                    s1ts = {}
                    es = {}
                    for gi, lset in enumerate(LGROUPS):
                        ne = len(lset)
                        s1t = ppool1.tile(
                            [P, ne, LT8], F32, tag=f"s1{gi}", name="s1t"
                        )
                        s1ts[gi] = s1t
                        for i, l in enumerate(lset):
                            for pr in range(kch // 2):
                                nc.tensor.matmul(
                                    s1t[:, i, 0:LT],
                                    w1sb[:, 2 * pr : 2 * pr + 2, ts(j, P)],
                                    xtb[:, 2 * pr : 2 * pr + 2, l, 0:LT],
                                    start=(pr == 0),
                                    stop=(pr == kch // 2 - 1),
                                    perf_mode=DR,
                                )
                        # exp + fused denominator partial (ACT accumulator)
                        e = epool.tile([P, ne, LT8], F32, tag=f"e{gi}", name="e")
                        es[gi] = e
                        nc.scalar.activation(
                            out=e[:, :, 0:LT], in_=s1t[:, :, 0:LT],
                            func=Exp, scale=1.0 / W1_SCALE,
                            accum_out=dcols[:, gi * jch + j : gi * jch + j + 1],
                        )
                    # s2 at per-l granularity (1-bank tiles, 3-deep) so the
                    # DVE product drains PSUM quickly
                    for gi, lset in enumerate(LGROUPS):
                        for i, l in enumerate(lset):
                            s2t = ppool2.tile([P, LT8], F32, tag="s2", name="s2t")
                            for pr in range(kch // 2):
                                nc.tensor.matmul(
                                    s2t[:, 0:LT],
                                    w2sb[:, 2 * pr : 2 * pr + 2, ts(j, P)],
                                    xtb[:, 2 * pr : 2 * pr + 2, l, 0:LT],
                                    start=(pr == 0),
                                    stop=(pr == kch // 2 - 1),
                                    perf_mode=DR,
                                )
                            prod = spool.tile([P, LT8], F32, tag="prod", name="prod")
                            # numer partial = sum_l (E/W2_SCALE) * s2 on DVE
                            nc.vector.scalar_tensor_tensor(
                                out=prod[:, 0:LT],
                                in0=es[gi][:, i, 0:LT], scalar=1.0 / W2_SCALE,
                                in1=s2t[:, 0:LT],
                                op0=mult, op1=mult,
                                accum_out=ncols[:, l * jch + j : l * jch + j + 1],
                            )
                    if bi + 1 < b and j >= 1:
                        # next batch's tanh split into half-k pieces over all
                        # 8 j-slots: a 2.27us whole-k tanh overshoots the
                        # ~1.16us per-j ACT slack and briefly stalls the PE
                        # (p-state recovery slices in the trace); the 1.36us
                        # + 0.93us pieces fit
                        if j == 1:
                            xtbs[bi + 1] = xtpool.tile(
                                [P, kch, LCH, LT8], FP8, tag="xt8", name="xtb"
                            )
                        k2 = (j - 1) // 2
                        if j % 2 == 1:
                            xraw = xpool.tile(
                                [P, LCH, LT], FP8, tag="xraw", name="xraw"
                            )
                            xraws[k2] = xraw
                            nc.sync.dma_start(out=xraw, in_=x8[bi + 1, k2])
                            nc.scalar.activation(
                                out=xtbs[bi + 1][:, k2, 0:3, 0:LT],
                                in_=xraw[:, 0:3], func=Tanh,
                            )
                        else:
                            nc.scalar.activation(
                                out=xtbs[bi + 1][:, k2, 3:LCH, 0:LT],
                                in_=xraws[k2][:, 3:LCH], func=Tanh,
                            )

                # batched softmax epilogue over all 9 class tiles
                dsum = eppool.tile([P, jch], F32, tag="dsum")
                nsA = eppool.tile([P, jch], F32, tag="nsA")
                nsB = eppool.tile([P, jch], F32, tag="nsB")
                nsC = eppool.tile([P, jch], F32, tag="nsC")
                nsum = eppool.tile([P, jch], F32, tag="nsum")
                recip = eppool.tile([P, jch], F32, tag="recip")
                tmp = eppool.tile([P, jch], F32, tag="tmp")
                nc.vector.scalar_tensor_tensor(
                    out=dsum, in0=dcols[:, 0:jch], scalar=1.0,
                    in1=dcols[:, jch : 2 * jch], op0=mult, op1=add,
                )
                nc.vector.scalar_tensor_tensor(
                    out=nsA, in0=ncols[:, 0:jch], scalar=1.0,
                    in1=ncols[:, jch : 2 * jch], op0=mult, op1=add,
                )
                nc.vector.scalar_tensor_tensor(
                    out=nsB, in0=ncols[:, 2 * jch : 3 * jch], scalar=1.0,
                    in1=ncols[:, 3 * jch : 4 * jch], op0=mult, op1=add,
                )
                nc.vector.scalar_tensor_tensor(
                    out=nsC, in0=nsA, scalar=1.0, in1=nsB, op0=mult, op1=add,
                )
                nc.vector.scalar_tensor_tensor(
                    out=nsum, in0=nsC, scalar=1.0,
                    in1=ncols[:, 4 * jch : 5 * jch], op0=mult, op1=add,
                )
                nc.vector.reciprocal(recip, dsum)
                nc.vector.scalar_tensor_tensor(
                    out=tmp, in0=nsum, scalar=1.0, in1=recip,
                    op0=mult, op1=mult,
                )
                # out = numer/denom + (b2 + mean-correction)[:, :, bi]
                nc.vector.scalar_tensor_tensor(
                    out=out_all[:, :, bi], in0=tmp, scalar=1.0,
                    in1=b2sb[:, :, bi], op0=mult, op1=add,
                )
                if bi == b - 1:
                    # single contiguous output DMA: 9 per-j DMAs serialized
                    # at ~600ns of HWDGE setup each in the kernel tail
                    nc.sync.dma_start(out=out[:], in_=out_all[:])

    nc.compile()
    return nc


_NC_CACHE = {}


def _get_nc():
    if "nc" not in _NC_CACHE:
        _NC_CACHE["nc"] = build_nc()
    return _NC_CACHE["nc"]


def make_in_maps(x, W1, W2, b2):
    """Host-side shard prep: pad C, pre-transpose + fp8-quantize weights,
    and fold the fp8 mean-correction into the bias."""
    x8 = (
        np.ascontiguousarray(np.asarray(x, dtype=np.float32))
        .reshape(B, KCH, P, L)
        .astype(FP8_NP)
    )
    # exact and quantized per-batch means of tanh(x) over l (host mirrors
    # the device pipeline: tanh of the fp8 x, then e4m3 rounding)
    xt32 = np.tanh(x8.astype(np.float32))                     # [B,KCH,P,L]
    xbar = xt32.mean(axis=3).reshape(B, D)                    # [B, D]
    xbar8 = (
        xt32.astype(FP8_NP).astype(np.float32).mean(axis=3).reshape(B, D)
    )

    def prep_w(W):
        Wp = np.zeros((C_PAD, D), dtype=np.float32)
        Wp[:C] = np.asarray(W, dtype=np.float32)
        return Wp

    W1p, W2p = prep_w(W1), prep_w(W2)
    w2q = (W2p * W2_SCALE).astype(FP8_NP).astype(np.float32) / W2_SCALE
    # out error from fp8 W2/xt is dominated by the near-uniform softmax
    # MEAN over l; replace that term with its exact value:
    corr = W2p @ xbar.T - w2q @ xbar8.T                       # [C_PAD, B]
    b2p = np.zeros((C_PAD,), dtype=np.float32)
    b2p[:C] = np.asarray(b2, dtype=np.float32)
    b2dfull = b2p[:, None] + corr                             # [C_PAD, B]

    in_maps = []
    for i in range(N_CORES):
        sl = slice(i * C_SH, (i + 1) * C_SH)
        w1t = np.ascontiguousarray(W1p[sl].T).reshape(KCH, P, C_SH)
        w2t = np.ascontiguousarray(W2p[sl].T).reshape(KCH, P, C_SH)
        b2d = np.ascontiguousarray(
            b2dfull[sl].reshape(JCH, P, B).transpose(1, 0, 2)
        )
        in_maps.append(
            {
                "x8": x8,
                "w1t": (w1t * W1_SCALE).astype(FP8_NP),
                "w2t": (w2t * W2_SCALE).astype(FP8_NP),
                "b2d": b2d,
            }
        )
    return in_maps


def gather_out(results):
    """results: list (per core) of {'out': [P, JCH, B]} -> full [B, C]."""
    parts = [
        np.transpose(np.asarray(r["out"], dtype=np.float32), (2, 1, 0)).reshape(B, C_SH)
        for r in results
    ]
    return np.concatenate(parts, axis=1)[:, :C]


def kernel(x, W1, W2, b2):
    nc = _get_nc()
    in_maps = make_in_maps(x, W1, W2, b2)
    res = run_bass_kernel_spmd(nc, in_maps, list(range(N_CORES)))
    return gather_out(res.results)


# revision 40
# speedup vs baseline: 1.1852x; 1.1852x over previous
"""CAML attention kernel for Trainium2 (8 NeuronCores, SPMD over classes).

Reference computation:
    xt      = tanh(x)                      # [B, D, L]
    scores  = einsum('cd,bdl->bcl', W1, xt)
    weights = softmax(scores, axis=l)
    weighted= einsum('bcl,bdl->bcd', weights, xt)
    out     = einsum('cd,bcd->bc', W2, weighted) + b2

Key identity: the final contraction commutes with the softmax weighted sum,
so with s2 = einsum('cd,bdl->bcl', W2, xt):
    out[b,c] = (sum_l exp(s1)*s2) / (sum_l exp(s1)) + b2
(|s1| <= 512*max|W1| ~ 13, so exp without max-subtraction is safe in fp32.)

v2: both matmuls run in fp8-e4m3 DoubleRow (2x PE throughput vs fp16 ->
~300us PE floor at 157 TF/s). Accuracy is recovered with a host-side
mean-correction folded into the bias:
    out ~= sum_l p_l s2q[c,l] + (W2 @ xbar_b - W2q @ xbar8_b)[c] + b2[c]
where xbar_b = mean_l tanh(x), xbar8_b = mean_l fp8(tanh(x)), both computed
on host (tiny C*D*B GEMM). The quantization error of W2 and of xt enters
out mostly through the (near-uniform-softmax) MEAN over L=2500 positions;
replacing that mean term with its exact value cuts rel err from ~1.5e-2 to
~6e-3 (sim), robust to device/host tanh table mismatch (which averages out
over L). The W1-side mean error cancels exactly via softmax shift
invariance, so no correction is needed there.

Other changes vs the fp16 baseline (638us -> 339us measured):
  - tanh (ACT) writes fp8 directly into the DoubleRow rhs layout
    [P, kch, lch, 512] (no DVE copy pass); x is shipped as fp8 (the extra
    tanh-input quantization costs ~1.5e-3 rel err, covered by the mean
    correction's margin, and shortens the cold-start x DMAs).
  - per-batch x loads + tanhs are spread across the previous batch's class
    loop (j in {1,3,5,7}), so ACT never bunches 9us of tanh at a batch
    boundary -- that bunching delayed exp, whose completion releases the
    s1 PSUM banks the PE needs next (measured 0.58us PE stalls + p-state
    recovery per occurrence before this change).
  - exp runs as 2 strided-AP instructions per (b,j): s1 PSUM tiles are
    (3,2) l-chunk groups (3+2 banks, single-buffered per tag -- the j->j+1
    serialization on exp has ~1.7us of slack), amortizing ACT PSUM-access
    latency and the 181ns accumulator-read of the fused denominator.
  - s2 stays at per-l 1-bank PSUM tiles (bufs=3) so the DVE product drains
    PSUM quickly: 5+3 = 8 banks exactly.
  - the softmax epilogue is batched per-b over all 9 class tiles ([P,9]
    ops) instead of per-(b,j) [P,1] ops.
  - one contiguous output DMA ([P, jch, b] layout) instead of 9 per-j
    DMAs, which serialized ~600ns of HWDGE setup each in the kernel tail.

Engine busy (pftrace, 339us total): PE 314us (93%, 1440 DoubleRow matmuls
in a zero-gap stream, LDWEIGHTS hidden by the PE reorder window), ACT
279us, DVE 240us. The PE matmul stream is the roofline; fp8-DR measures
2x bf16 (not 4x) on trn2. Remaining overhead is the ~7.7us runtime
preamble, the 9.1us serial batch-0 tanh chain on ACT (cold, fed at
~10.8us by the first fp8 x chunk - batch-0 x DMAs lead the queue, weights
ride under the tanh chain), and ~12us of tail/teardown; starting the PE
earlier via split/dual-queue cold loads was measured a wash (stalls just
move into the stream).

Sharding: C padded 8930 -> 9216 = 8 cores * 1152; weights row-sharded per
core, x replicated. Zero-padded weight rows give out=0 there, discarded on
the host after gathering.
"""

import numpy as np
import ml_dtypes

import concourse.bacc as bacc
import concourse.tile as tile
from concourse import mybir
from concourse.bass import ts
from concourse.bass_utils import run_bass_kernel_spmd

B, D, L, C = 8, 512, 2500, 8930
N_CORES = 8
P = 128

C_PAD = 9216                 # next multiple of 8*128 above C
C_SH = C_PAD // N_CORES      # 1152 classes per core
KCH = D // P                 # 4 contraction chunks (pairs for DoubleRow)
JCH = C_SH // P              # 9 class chunks per core
LCH = 5                      # l chunks
LT = L // LCH                # 500 columns per matmul (fits one PSUM bank)
LT8 = 512                    # l-chunk stride: bank-sized, 16B-aligned for fp8 rhs
LGROUPS = [(0, 1, 2), (3, 4)]  # l-chunks fused per s1-PSUM tile / exp instr

F32 = mybir.dt.float32
F16 = mybir.dt.float16
FP8 = mybir.dt.float8e4
FP8_NP = mybir.dt.np(mybir.dt.float8e4)  # ml_dtypes.float8_e4m3

# fp8 weights are scaled into e4m3's normal range (min normal 2^-6 vs
# |W| <= 0.0252); exp's input scale and the product's scalar compensate.
W1_SCALE = 16.0
W2_SCALE = 32.0

FP8_S1 = True  # legacy knob read by test.py; both paths are fp8 here


def build_nc(b=B, kch=KCH, jch=JCH):
    """Emit the per-core program. All cores run the same NEFF (SPMD)."""
    nc = bacc.Bacc("TRN2", target_bir_lowering=False, debug=False)

    x8 = nc.dram_tensor("x8", [b, kch, P, L], FP8, kind="ExternalInput")
    w1t = nc.dram_tensor("w1t", [kch, P, jch * P], FP8, kind="ExternalInput")
    w2t = nc.dram_tensor("w2t", [kch, P, jch * P], FP8, kind="ExternalInput")
    b2d = nc.dram_tensor("b2d", [P, jch, b], F32, kind="ExternalInput")
    out = nc.dram_tensor("out", [P, jch, b], F32, kind="ExternalOutput")

    Exp = mybir.ActivationFunctionType.Exp
    Tanh = mybir.ActivationFunctionType.Tanh
    mult = mybir.AluOpType.mult
    add = mybir.AluOpType.add
    DR = mybir.MatmulPerfMode.DoubleRow

    with tile.TileContext(nc) as tc:
        with (
            tc.tile_pool(name="wts", bufs=1) as wpool,
            tc.tile_pool(name="xraw", bufs=8) as xpool,
            tc.tile_pool(name="xt8", bufs=2) as xtpool,
            tc.tile_pool(name="ps1", bufs=1, space="PSUM") as ppool1,
            tc.tile_pool(name="ps2", bufs=3, space="PSUM") as ppool2,
            tc.tile_pool(name="etile", bufs=2) as epool,
            tc.tile_pool(name="scratch", bufs=3) as spool,
            tc.tile_pool(name="cols", bufs=2) as cpool,
            tc.tile_pool(name="epi", bufs=2) as eppool,
            tc.tile_pool(name="outp", bufs=1) as opool,
        ):
            # one fast HWDGE queue, ordered by first consumption
            w1sb = wpool.tile([P, kch, jch * P], FP8)
            w2sb = wpool.tile([P, kch, jch * P], FP8)
            b2sb = wpool.tile([P, jch, b], F32)
            out_all = opool.tile([P, jch, b], F32)

            # load fp8 x, tanh straight to fp8 in the DoubleRow rhs
            # layout [P, kch, LCH, LT8] (l-chunk stride 512 cols)
            xtbs = {}

            def emit_load(bload, k):
                xraw = xpool.tile([P, LCH, LT], FP8, tag="xraw")
                nc.sync.dma_start(out=xraw, in_=x8[bload, k])
                nc.scalar.activation(
                    out=xtbs[bload][:, k, :, 0:LT], in_=xraw, func=Tanh
                )

            # cold start: batch 0 up front; later batches prefetch spread
            # across the previous batch's class loop so ACT never bunches
            # tanhs at a batch boundary (which stalls the PE on PSUM reuse)
            xtbs[0] = xtpool.tile(
                [P, kch, LCH, LT8], FP8, tag="xt8", name="xtb"
            )
            # (starting the PE earlier via split/dual-queue cold loads was
            # measured a wash: the tanh-wait stalls just move into the
            # matmul stream -- the dependency-driven schedule below is
            # already cold-optimal given the 9.1us serial tanh chain)
            for k in range(kch):
                emit_load(0, k)
            for k in range(kch):
                nc.sync.dma_start(out=w1sb[:, k], in_=w1t[k])
            for k in range(kch):
                nc.sync.dma_start(out=w2sb[:, k], in_=w2t[k])
            nc.sync.dma_start(out=b2sb, in_=b2d[:])

            for bi in range(b):
                xtb = xtbs[bi]
                xraws = {}
                dcols = cpool.tile([P, 2 * jch], F32, tag="dcols")
                ncols = cpool.tile([P, LCH * jch], F32, tag="ncols")
                for j in range(jch):
                    # s1 tiles are single-buffered per group tag (3+2 banks);
                    # the j->j+1 matmul reuse serializes on exp(j), which
                    # finishes ~1.7us before the PE needs the banks back
                    s1ts = {}
                    es = {}
                    for gi, lset in enumerate(LGROUPS):
                        ne = len(lset)
                        s1t = ppool1.tile(
                            [P, ne, LT8], F32, tag=f"s1{gi}", name="s1t"
                        )
                        s1ts[gi] = s1t
                        for i, l in enumerate(lset):
                            for pr in range(kch // 2):
                                nc.tensor.matmul(
                                    s1t[:, i, 0:LT],
                                    w1sb[:, 2 * pr : 2 * pr + 2, ts(j, P)],
                                    xtb[:, 2 * pr : 2 * pr + 2, l, 0:LT],
                                    start=(pr == 0),
                                    stop=(pr == kch // 2 - 1),
                                    perf_mode=DR,
                                )
                        # exp + fused denominator partial (ACT accumulator)
                        e = epool.tile([P, ne, LT8], F32, tag=f"e{gi}", name="e")
                        es[gi] = e
                        nc.scalar.activation(
                            out=e[:, :, 0:LT], in_=s1t[:, :, 0:LT],
                            func=Exp, scale=1.0 / W1_SCALE,
                            accum_out=dcols[:, gi * jch + j : gi * jch + j + 1],
                        )
                    # s2 at per-l granularity (1-bank tiles, 3-deep) so the
                    # DVE product drains PSUM quickly
                    for gi, lset in enumerate(LGROUPS):
                        for i, l in enumerate(lset):
                            s2t = ppool2.tile([P, LT8], F32, tag="s2", name="s2t")
                            for pr in range(kch // 2):
                                nc.tensor.matmul(
                                    s2t[:, 0:LT],
                                    w2sb[:, 2 * pr : 2 * pr + 2, ts(j, P)],
                                    xtb[:, 2 * pr : 2 * pr + 2, l, 0:LT],
                                    start=(pr == 0),
                                    stop=(pr == kch // 2 - 1),
                                    perf_mode=DR,
                                )
                            prod = spool.tile([P, LT8], F32, tag="prod", name="prod")
                            # numer partial = sum_l (E/W2_SCALE) * s2 on DVE
                            nc.vector.scalar_tensor_tensor(
                                out=prod[:, 0:LT],
                                in0=es[gi][:, i, 0:LT], scalar=1.0 / W2_SCALE,
                                in1=s2t[:, 0:LT],
                                op0=mult, op1=mult,
                                accum_out=ncols[:, l * jch + j : l * jch + j + 1],
                            )
                    if bi + 1 < b and j >= 1:
                        # next batch's tanh split into half-k pieces over all
                        # 8 j-slots: a 2.27us whole-k tanh overshoots the
                        # ~1.16us per-j ACT slack and briefly stalls the PE
                        # (p-state recovery slices in the trace); the 1.36us
                        # + 0.93us pieces fit
                        if j == 1:
                            xtbs[bi + 1] = xtpool.tile(
                                [P, kch, LCH, LT8], FP8, tag="xt8", name="xtb"
                            )
                        k2 = (j - 1) // 2
                        if j % 2 == 1:
                            xraw = xpool.tile(
                                [P, LCH, LT], FP8, tag="xraw", name="xraw"
                            )
                            xraws[k2] = xraw
                            nc.sync.dma_start(out=xraw, in_=x8[bi + 1, k2])
                            nc.scalar.activation(
                                out=xtbs[bi + 1][:, k2, 0:3, 0:LT],
                                in_=xraw[:, 0:3], func=Tanh,
                            )
                        else:
                            nc.scalar.activation(
                                out=xtbs[bi + 1][:, k2, 3:LCH, 0:LT],
                                in_=xraws[k2][:, 3:LCH], func=Tanh,
                            )

                # batched softmax epilogue over all 9 class tiles
                dsum = eppool.tile([P, jch], F32, tag="dsum")
                nsA = eppool.tile([P, jch], F32, tag="nsA")
                nsB = eppool.tile([P, jch], F32, tag="nsB")
                nsC = eppool.tile([P, jch], F32, tag="nsC")
                nsum = eppool.tile([P, jch], F32, tag="nsum")
                recip = eppool.tile([P, jch], F32, tag="recip")
                tmp = eppool.tile([P, jch], F32, tag="tmp")
                nc.vector.scalar_tensor_tensor(
                    out=dsum, in0=dcols[:, 0:jch], scalar=1.0,
                    in1=dcols[:, jch : 2 * jch], op0=mult, op1=add,
                )
                nc.vector.scalar_tensor_tensor(
                    out=nsA, in0=ncols[:, 0:jch], scalar=1.0,
                    in1=ncols[:, jch : 2 * jch], op0=mult, op1=add,
                )
                nc.vector.scalar_tensor_tensor(
                    out=nsB, in0=ncols[:, 2 * jch : 3 * jch], scalar=1.0,
                    in1=ncols[:, 3 * jch : 4 * jch], op0=mult, op1=add,
                )
                nc.vector.scalar_tensor_tensor(
                    out=nsC, in0=nsA, scalar=1.0, in1=nsB, op0=mult, op1=add,
                )
                nc.vector.scalar_tensor_tensor(
                    out=nsum, in0=nsC, scalar=1.0,
                    in1=ncols[:, 4 * jch : 5 * jch], op0=mult, op1=add,
                )
                nc.vector.reciprocal(recip, dsum)
                nc.vector.scalar_tensor_tensor(
                    out=tmp, in0=nsum, scalar=1.0, in1=recip,
                    op0=mult, op1=mult,
                )
                # out = numer/denom + (b2 + mean-correction)[:, :, bi]
                nc.vector.scalar_tensor_tensor(
                    out=out_all[:, :, bi], in0=tmp, scalar=1.0,
                    in1=b2sb[:, :, bi], op0=mult, op1=add,
                )
                if bi == b - 1:
                    # single contiguous output DMA: 9 per-j DMAs serialized
                    # at ~600ns of HWDGE setup each in the kernel tail
                    nc.sync.dma_start(out=out[:], in_=out_all[:])

    nc.compile()
    return nc


_NC_CACHE = {}


def _get_nc():
    if "nc" not in _NC_CACHE:
        _NC_CACHE["nc"] = build_nc()
    return _NC_CACHE["nc"]


def make_in_maps(x, W1, W2, b2):
    """Host-side shard prep: pad C, pre-transpose + fp8-quantize weights,
    and fold the fp8 mean-correction into the bias."""
    x8 = (
        np.ascontiguousarray(np.asarray(x, dtype=np.float32))
        .reshape(B, KCH, P, L)
        .astype(FP8_NP)
    )
    # exact and quantized per-batch means of tanh(x) over l (host mirrors
    # the device pipeline: tanh of the fp8 x, then e4m3 rounding)
    xt32 = np.tanh(x8.astype(np.float32))                     # [B,KCH,P,L]
    xbar = xt32.mean(axis=3).reshape(B, D)                    # [B, D]
    xbar8 = (
        xt32.astype(FP8_NP).astype(np.float32).mean(axis=3).reshape(B, D)
    )

    def prep_w(W):
        Wp = np.zeros((C_PAD, D), dtype=np.float32)
        Wp[:C] = np.asarray(W, dtype=np.float32)
        return Wp

    W1p, W2p = prep_w(W1), prep_w(W2)
    w2q = (W2p * W2_SCALE).astype(FP8_NP).astype(np.float32) / W2_SCALE
    # out error from fp8 W2/xt is dominated by the near-uniform softmax
    # MEAN over l; replace that term with its exact value:
    corr = W2p @ xbar.T - w2q @ xbar8.T                       # [C_PAD, B]
    b2p = np.zeros((C_PAD,), dtype=np.float32)
    b2p[:C] = np.asarray(b2, dtype=np.float32)
    b2dfull = b2p[:, None] + corr                             # [C_PAD, B]

    in_maps = []
    for i in range(N_CORES):
        sl = slice(i * C_SH, (i + 1) * C_SH)
        w1t = np.ascontiguousarray(W1p[sl].T).reshape(KCH, P, C_SH)
        w2t = np.ascontiguousarray(W2p[sl].T).reshape(KCH, P, C_SH)
        b2d = np.ascontiguousarray(
            b2dfull[sl].reshape(JCH, P, B).transpose(1, 0, 2)
        )
        in_maps.append(
            {
                "x8": x8,
                "w1t": (w1t * W1_SCALE).astype(FP8_NP),
                "w2t": (w2t * W2_SCALE).astype(FP8_NP),
                "b2d": b2d,
            }
        )
    return in_maps


def gather_out(results):
    """results: list (per core) of {'out': [P, JCH, B]} -> full [B, C]."""
    parts = [
        np.transpose(np.asarray(r["out"], dtype=np.float32), (2, 1, 0)).reshape(B, C_SH)
        for r in results
    ]
    return np.concatenate(parts, axis=1)[:, :C]


def kernel(x, W1, W2, b2):
    nc = _get_nc()
    in_maps = make_in_maps(x, W1, W2, b2)
    res = run_bass_kernel_spmd(nc, in_maps, list(range(N_CORES)))
    return gather_out(res.results)


# revision 41
# speedup vs baseline: 1.1970x; 1.0100x over previous
"""CAML attention kernel for Trainium2 (8 NeuronCores, SPMD over classes).

Reference computation:
    xt      = tanh(x)                      # [B, D, L]
    scores  = einsum('cd,bdl->bcl', W1, xt)
    weights = softmax(scores, axis=l)
    weighted= einsum('bcl,bdl->bcd', weights, xt)
    out     = einsum('cd,bcd->bc', W2, weighted) + b2

Key identity: the final contraction commutes with the softmax weighted sum,
so with s2 = einsum('cd,bdl->bcl', W2, xt):
    out[b,c] = (sum_l exp(s1)*s2) / (sum_l exp(s1)) + b2
(|s1| <= 512*max|W1| ~ 13, so exp without max-subtraction is safe in fp32.)

v2: both matmuls run in fp8-e4m3 DoubleRow (2x PE throughput vs fp16 ->
~300us PE floor at 157 TF/s). Accuracy is recovered with a host-side
mean-correction folded into the bias:
    out ~= sum_l p_l s2q[c,l] + (W2 @ xbar_b - W2q @ xbar8_b)[c] + b2[c]
where xbar_b = mean_l tanh(x), xbar8_b = mean_l fp8(tanh(x)), both computed
on host (tiny C*D*B GEMM). The quantization error of W2 and of xt enters
out mostly through the (near-uniform-softmax) MEAN over L=2500 positions;
replacing that mean term with its exact value cuts rel err from ~1.5e-2 to
~6e-3 (sim), robust to device/host tanh table mismatch (which averages out
over L). The W1-side mean error cancels exactly via softmax shift
invariance, so no correction is needed there.

Other changes vs the fp16 baseline (638us -> 339us measured):
  - tanh (ACT) writes fp8 directly into the DoubleRow rhs layout
    [P, kch, lch, 512] (no DVE copy pass); x is shipped as fp8 (the extra
    tanh-input quantization costs ~1.5e-3 rel err, covered by the mean
    correction's margin, and shortens the cold-start x DMAs).
  - per-batch x loads + tanhs are spread across the previous batch's class
    loop (j in {1,3,5,7}), so ACT never bunches 9us of tanh at a batch
    boundary -- that bunching delayed exp, whose completion releases the
    s1 PSUM banks the PE needs next (measured 0.58us PE stalls + p-state
    recovery per occurrence before this change).
  - exp runs as 2 strided-AP instructions per (b,j): s1 PSUM tiles are
    (3,2) l-chunk groups (3+2 banks, single-buffered per tag -- the j->j+1
    serialization on exp has ~1.7us of slack), amortizing ACT PSUM-access
    latency and the 181ns accumulator-read of the fused denominator.
  - s2 stays at per-l 1-bank PSUM tiles (bufs=3) so the DVE product drains
    PSUM quickly: 5+3 = 8 banks exactly.
  - the softmax epilogue is batched per-b over all 9 class tiles ([P,9]
    ops) instead of per-(b,j) [P,1] ops.
  - one contiguous output DMA ([P, jch, b] layout) instead of 9 per-j
    DMAs, which serialized ~600ns of HWDGE setup each in the kernel tail.

Engine busy (pftrace, 339us total): PE 314us (93%, 1440 DoubleRow matmuls
in a zero-gap stream, LDWEIGHTS hidden by the PE reorder window), ACT
279us, DVE 240us. The PE matmul stream is the roofline; fp8-DR measures
2x bf16 (not 4x) on trn2. Remaining overhead is the ~7.7us runtime
preamble, the 9.1us serial batch-0 tanh chain on ACT (cold, fed at
~10.8us by the first fp8 x chunk - batch-0 x DMAs lead the queue, weights
ride under the tanh chain), and ~12us of tail/teardown; starting the PE
earlier via split/dual-queue cold loads was measured a wash (stalls just
move into the stream).

Sharding: C padded 8930 -> 9216 = 8 cores * 1152; weights row-sharded per
core, x replicated. Zero-padded weight rows give out=0 there, discarded on
the host after gathering.
"""

import numpy as np
import ml_dtypes

import concourse.bacc as bacc
import concourse.tile as tile
from concourse import mybir
from concourse.bass import ts
from concourse.bass_utils import run_bass_kernel_spmd

B, D, L, C = 8, 512, 2500, 8930
N_CORES = 8
P = 128

C_PAD = 9216                 # next multiple of 8*128 above C
C_SH = C_PAD // N_CORES      # 1152 classes per core
KCH = D // P                 # 4 contraction chunks (pairs for DoubleRow)
JCH = C_SH // P              # 9 class chunks per core
LCH = 5                      # l chunks
LT = L // LCH                # 500 columns per matmul (fits one PSUM bank)
LT8 = 512                    # l-chunk stride: bank-sized, 16B-aligned for fp8 rhs
LGROUPS = [(0, 1, 2), (3, 4)]  # l-chunks fused per s1-PSUM tile / exp instr

F32 = mybir.dt.float32
F16 = mybir.dt.float16
FP8 = mybir.dt.float8e4
FP8_NP = mybir.dt.np(mybir.dt.float8e4)  # ml_dtypes.float8_e4m3

# fp8 weights are scaled into e4m3's normal range (min normal 2^-6 vs
# |W| <= 0.0252); exp's input scale and the product's scalar compensate.
W1_SCALE = 16.0
W2_SCALE = 32.0

FP8_S1 = True  # legacy knob read by test.py; both paths are fp8 here


def build_nc(b=B, kch=KCH, jch=JCH):
    """Emit the per-core program. All cores run the same NEFF (SPMD)."""
    nc = bacc.Bacc("TRN2", target_bir_lowering=False, debug=False)

    x8 = nc.dram_tensor("x8", [b, kch, P, L], FP8, kind="ExternalInput")
    w1t = nc.dram_tensor("w1t", [kch, P, jch * P], FP8, kind="ExternalInput")
    w2t = nc.dram_tensor("w2t", [kch, P, jch * P], FP8, kind="ExternalInput")
    b2d = nc.dram_tensor("b2d", [P, jch, b], F32, kind="ExternalInput")
    out = nc.dram_tensor("out", [P, jch, b], F32, kind="ExternalOutput")

    Exp = mybir.ActivationFunctionType.Exp
    Tanh = mybir.ActivationFunctionType.Tanh
    mult = mybir.AluOpType.mult
    add = mybir.AluOpType.add
    DR = mybir.MatmulPerfMode.DoubleRow

    with tile.TileContext(nc) as tc:
        with (
            tc.tile_pool(name="wts", bufs=1) as wpool,
            tc.tile_pool(name="xraw", bufs=8) as xpool,
            tc.tile_pool(name="xt8", bufs=2) as xtpool,
            tc.tile_pool(name="ps1", bufs=1, space="PSUM") as ppool1,
            tc.tile_pool(name="ps2", bufs=3, space="PSUM") as ppool2,
            tc.tile_pool(name="etile", bufs=2) as epool,
            tc.tile_pool(name="scratch", bufs=3) as spool,
            tc.tile_pool(name="cols", bufs=2) as cpool,
            tc.tile_pool(name="epi", bufs=2) as eppool,
            tc.tile_pool(name="outp", bufs=1) as opool,
        ):
            # one fast HWDGE queue, ordered by first consumption
            w1sb = wpool.tile([P, kch, jch * P], FP8)
            w2sb = wpool.tile([P, kch, jch * P], FP8)
            b2sb = wpool.tile([P, jch, b], F32)
            out_all = opool.tile([P, jch, b], F32)

            # PE clock warm-up: junk matmuls on a zeroed tile keep the PE
            # continuously busy through the ~15us cold DMA+tanh phase so
            # DVFS reaches full clock before the real stream begins (the
            # first ~100 real matmuls otherwise average 254ns vs the 211ns
            # warm rate -- p-state ramp)
            warm_sb = wpool.tile([P, 640], FP8, name="warm_sb")
            nc.gpsimd.memset(warm_sb, 0.0)
            warm_ps = ppool2.tile([P, LT8], F32, tag="s2", name="warm_ps")
            for _ in range(44):
                nc.tensor.matmul(
                    warm_ps[:, 0:LT], warm_sb[:, 0:P], warm_sb[:, 0:LT],
                    start=True, stop=True,
                )

            # load fp8 x, tanh straight to fp8 in the DoubleRow rhs
            # layout [P, kch, LCH, LT8] (l-chunk stride 512 cols)
            xtbs = {}

            def emit_load(bload, k):
                xraw = xpool.tile([P, LCH, LT], FP8, tag="xraw")
                nc.sync.dma_start(out=xraw, in_=x8[bload, k])
                nc.scalar.activation(
                    out=xtbs[bload][:, k, :, 0:LT], in_=xraw, func=Tanh
                )

            # cold start: batch 0 up front; later batches prefetch spread
            # across the previous batch's class loop so ACT never bunches
            # tanhs at a batch boundary (which stalls the PE on PSUM reuse)
            xtbs[0] = xtpool.tile(
                [P, kch, LCH, LT8], FP8, tag="xt8", name="xtb"
            )
            # (starting the PE earlier via split/dual-queue cold loads was
            # measured a wash: the tanh-wait stalls just move into the
            # matmul stream -- the dependency-driven schedule below is
            # already cold-optimal given the 9.1us serial tanh chain)
            for k in range(kch):
                emit_load(0, k)
            for k in range(kch):
                nc.sync.dma_start(out=w1sb[:, k], in_=w1t[k])
            for k in range(kch):
                nc.sync.dma_start(out=w2sb[:, k], in_=w2t[k])
            nc.sync.dma_start(out=b2sb, in_=b2d[:])

            for bi in range(b):
                xtb = xtbs[bi]
                xraws = {}
                dcols = cpool.tile([P, 2 * jch], F32, tag="dcols")
                ncols = cpool.tile([P, LCH * jch], F32, tag="ncols")
                for j in range(jch):
                    # s1 tiles are single-buffered per group tag (3+2 banks);
                    # the j->j+1 matmul reuse serializes on exp(j), which
                    # finishes ~1.7us before the PE needs the banks back
                    s1ts = {}
                    es = {}
                    for gi, lset in enumerate(LGROUPS):
                        ne = len(lset)
                        s1t = ppool1.tile(
                            [P, ne, LT8], F32, tag=f"s1{gi}", name="s1t"
                        )
                        s1ts[gi] = s1t
                        for i, l in enumerate(lset):
                            for pr in range(kch // 2):
                                nc.tensor.matmul(
                                    s1t[:, i, 0:LT],
                                    w1sb[:, 2 * pr : 2 * pr + 2, ts(j, P)],
                                    xtb[:, 2 * pr : 2 * pr + 2, l, 0:LT],
                                    start=(pr == 0),
                                    stop=(pr == kch // 2 - 1),
                                    perf_mode=DR,
                                )
                        # exp + fused denominator partial (ACT accumulator)
                        e = epool.tile([P, ne, LT8], F32, tag=f"e{gi}", name="e")
                        es[gi] = e
                        nc.scalar.activation(
                            out=e[:, :, 0:LT], in_=s1t[:, :, 0:LT],
                            func=Exp, scale=1.0 / W1_SCALE,
                            accum_out=dcols[:, gi * jch + j : gi * jch + j + 1],
                        )
                    # s2 at per-l granularity (1-bank tiles, 3-deep) so the
                    # DVE product drains PSUM quickly
                    for gi, lset in enumerate(LGROUPS):
                        for i, l in enumerate(lset):
                            s2t = ppool2.tile([P, LT8], F32, tag="s2", name="s2t")
                            for pr in range(kch // 2):
                                nc.tensor.matmul(
                                    s2t[:, 0:LT],
                                    w2sb[:, 2 * pr : 2 * pr + 2, ts(j, P)],
                                    xtb[:, 2 * pr : 2 * pr + 2, l, 0:LT],
                                    start=(pr == 0),
                                    stop=(pr == kch // 2 - 1),
                                    perf_mode=DR,
                                )
                            prod = spool.tile([P, LT8], F32, tag="prod", name="prod")
                            # numer partial = sum_l (E/W2_SCALE) * s2 on DVE
                            nc.vector.scalar_tensor_tensor(
                                out=prod[:, 0:LT],
                                in0=es[gi][:, i, 0:LT], scalar=1.0 / W2_SCALE,
                                in1=s2t[:, 0:LT],
                                op0=mult, op1=mult,
                                accum_out=ncols[:, l * jch + j : l * jch + j + 1],
                            )
                    if bi + 1 < b and j >= 1:
                        # next batch's tanh split into half-k pieces over all
                        # 8 j-slots: a 2.27us whole-k tanh overshoots the
                        # ~1.16us per-j ACT slack and briefly stalls the PE
                        # (p-state recovery slices in the trace); the 1.36us
                        # + 0.93us pieces fit
                        if j == 1:
                            xtbs[bi + 1] = xtpool.tile(
                                [P, kch, LCH, LT8], FP8, tag="xt8", name="xtb"
                            )
                        k2 = (j - 1) // 2
                        if j % 2 == 1:
                            xraw = xpool.tile(
                                [P, LCH, LT], FP8, tag="xraw", name="xraw"
                            )
                            xraws[k2] = xraw
                            nc.sync.dma_start(out=xraw, in_=x8[bi + 1, k2])
                            nc.scalar.activation(
                                out=xtbs[bi + 1][:, k2, 0:3, 0:LT],
                                in_=xraw[:, 0:3], func=Tanh,
                            )
                        else:
                            nc.scalar.activation(
                                out=xtbs[bi + 1][:, k2, 3:LCH, 0:LT],
                                in_=xraws[k2][:, 3:LCH], func=Tanh,
                            )

                # batched softmax epilogue over all 9 class tiles
                dsum = eppool.tile([P, jch], F32, tag="dsum")
                nsA = eppool.tile([P, jch], F32, tag="nsA")
                nsB = eppool.tile([P, jch], F32, tag="nsB")
                nsC = eppool.tile([P, jch], F32, tag="nsC")
                nsum = eppool.tile([P, jch], F32, tag="nsum")
                recip = eppool.tile([P, jch], F32, tag="recip")
                tmp = eppool.tile([P, jch], F32, tag="tmp")
                nc.vector.scalar_tensor_tensor(
                    out=dsum, in0=dcols[:, 0:jch], scalar=1.0,
                    in1=dcols[:, jch : 2 * jch], op0=mult, op1=add,
                )
                nc.vector.scalar_tensor_tensor(
                    out=nsA, in0=ncols[:, 0:jch], scalar=1.0,
                    in1=ncols[:, jch : 2 * jch], op0=mult, op1=add,
                )
                nc.vector.scalar_tensor_tensor(
                    out=nsB, in0=ncols[:, 2 * jch : 3 * jch], scalar=1.0,
                    in1=ncols[:, 3 * jch : 4 * jch], op0=mult, op1=add,
                )
                nc.vector.scalar_tensor_tensor(
                    out=nsC, in0=nsA, scalar=1.0, in1=nsB, op0=mult, op1=add,
                )
                nc.vector.scalar_tensor_tensor(
                    out=nsum, in0=nsC, scalar=1.0,
                    in1=ncols[:, 4 * jch : 5 * jch], op0=mult, op1=add,
                )
                nc.vector.reciprocal(recip, dsum)
                nc.vector.scalar_tensor_tensor(
                    out=tmp, in0=nsum, scalar=1.0, in1=recip,
                    op0=mult, op1=mult,
                )
                # out = numer/denom + (b2 + mean-correction)[:, :, bi]
                nc.vector.scalar_tensor_tensor(
                    out=out_all[:, :, bi], in0=tmp, scalar=1.0,
                    in1=b2sb[:, :, bi], op0=mult, op1=add,
                )
                if bi == b - 1:
                    # single contiguous output DMA: 9 per-j DMAs serialized
                    # at ~600ns of HWDGE setup each in the kernel tail
                    nc.sync.dma_start(out=out[:], in_=out_all[:])

    nc.compile()
    return nc


_NC_CACHE = {}


def _get_nc():
    if "nc" not in _NC_CACHE:
        _NC_CACHE["nc"] = build_nc()
    return _NC_CACHE["nc"]


def make_in_maps(x, W1, W2, b2):
    """Host-side shard prep: pad C, pre-transpose + fp8-quantize weights,
    and fold the fp8 mean-correction into the bias."""
    x8 = (
        np.ascontiguousarray(np.asarray(x, dtype=np.float32))
        .reshape(B, KCH, P, L)
        .astype(FP8_NP)
    )
    # exact and quantized per-batch means of tanh(x) over l (host mirrors
    # the device pipeline: tanh of the fp8 x, then e4m3 rounding)
    xt32 = np.tanh(x8.astype(np.float32))                     # [B,KCH,P,L]
    xbar = xt32.mean(axis=3).reshape(B, D)                    # [B, D]
    xbar8 = (
        xt32.astype(FP8_NP).astype(np.float32).mean(axis=3).reshape(B, D)
    )

    def prep_w(W):
        Wp = np.zeros((C_PAD, D), dtype=np.float32)
        Wp[:C] = np.asarray(W, dtype=np.float32)
        return Wp

    W1p, W2p = prep_w(W1), prep_w(W2)
    w2q = (W2p * W2_SCALE).astype(FP8_NP).astype(np.float32) / W2_SCALE
    # out error from fp8 W2/xt is dominated by the near-uniform softmax
    # MEAN over l; replace that term with its exact value:
    corr = W2p @ xbar.T - w2q @ xbar8.T                       # [C_PAD, B]
    b2p = np.zeros((C_PAD,), dtype=np.float32)
    b2p[:C] = np.asarray(b2, dtype=np.float32)
    b2dfull = b2p[:, None] + corr                             # [C_PAD, B]

    in_maps = []
    for i in range(N_CORES):
        sl = slice(i * C_SH, (i + 1) * C_SH)
        w1t = np.ascontiguousarray(W1p[sl].T).reshape(KCH, P, C_SH)
        w2t = np.ascontiguousarray(W2p[sl].T).reshape(KCH, P, C_SH)
        b2d = np.ascontiguousarray(
            b2dfull[sl].reshape(JCH, P, B).transpose(1, 0, 2)
        )
        in_maps.append(
            {
                "x8": x8,
                "w1t": (w1t * W1_SCALE).astype(FP8_NP),
                "w2t": (w2t * W2_SCALE).astype(FP8_NP),
                "b2d": b2d,
            }
        )
    return in_maps


def gather_out(results):
    """results: list (per core) of {'out': [P, JCH, B]} -> full [B, C]."""
    parts = [
        np.transpose(np.asarray(r["out"], dtype=np.float32), (2, 1, 0)).reshape(B, C_SH)
        for r in results
    ]
    return np.concatenate(parts, axis=1)[:, :C]


def kernel(x, W1, W2, b2):
    nc = _get_nc()
    in_maps = make_in_maps(x, W1, W2, b2)
    res = run_bass_kernel_spmd(nc, in_maps, list(range(N_CORES)))
    return gather_out(res.results)
